# revision 27
# baseline (speedup 1.0000x reference)
"""Chunked cross-attention (RETRO-style) TRN2 Bass kernel.

Sharding: pure data-parallel over (batch, chunk-block): 8 cores =
4 batches x 2 chunk-halves (16 chunks each). Every (batch, chunk) pair
attends only to its own retrieved neighbors, and all projections are
row-wise, so there is no cross-core communication at all.

Per-core pipeline (all matmul operands bf16, fp32 PSUM accumulation,
fp32 layernorm/softmax statistics):
  phase 1: LN(h rows) -> PE-transpose -> Q^T = Wq^T @ hn^T  (feature-major)
  per chunk-pair group (8 groups x 2 chunks):
    K^T = Wk^T @ e^T   (feature-major, e^T pre-transposed on host)
    V   = e @ Wv       (token-major, e^T as stationary operand)
    scores[c,h] = Q_h^T.T @ K_h^T  (both chunks packed into 128 psum
                  partitions via PE quadrant placement)
    exp + row-sum fused on ACT (accum_out); probs = exp * recip(sum)
    probs^T via PE transpose; attnout = V.T @ probs^T (feature-major)
    out2 = attnout.T @ Wo (token-major) + h residual -> DMA out
gamma/beta are folded into Wq/bq on the host; bq/bk are fused into the
PSUM->SBUF copies; bv/bo paths are emitted only if nonzero.
"""

import os
import sys

import numpy as np

for _p in ("/opt/trn_rl_repo", "/root/.axon_site/_ro/trn_rl_repo"):
    if os.path.isdir(_p) and _p not in sys.path:
        sys.path.append(_p)

import ml_dtypes  # noqa: E402

import concourse.bass as bass  # noqa: E402
import concourse.bacc as bacc  # noqa: E402
import concourse.mybir as mybir  # noqa: E402
import concourse.tile as tile  # noqa: E402
from concourse.masks import make_identity  # noqa: E402

BF16 = mybir.dt.bfloat16
F32 = mybir.dt.float32
AF = mybir.ActivationFunctionType
ALU = mybir.AluOpType
NP_BF16 = ml_dtypes.bfloat16

D_MODEL = 1024
N_HEADS = 16
D_K = 64
HD = N_HEADS * D_K  # 1024
CHUNK_LEN = 64
SEQ = 2048
BATCH = 4
N_CORES = 8
CH_PER_CORE = 16
TOK = CH_PER_CORE * CHUNK_LEN  # 1024 q-side tokens per core
NGROUP = 8                     # chunk-pair groups per core
GTOK = 128                     # q tokens per group (2 chunks)
KVT = 512                      # kv tokens per group (2 chunks * 2 * 128)
KD = D_MODEL // 128            # 8 contraction tiles
EPS = 1e-5
SCALE = 1.0 / float(np.sqrt(D_K))


def _build(add_bv: bool, add_bo: bool) -> bass.Bass:
    nc = bacc.Bacc("TRN2", target_bir_lowering=False, debug=False,
                   num_devices=N_CORES)

    hslab_d = nc.dram_tensor("hslab", [TOK, D_MODEL], F32,
                             kind="ExternalInput").ap()
    eT_d = nc.dram_tensor("eT", [D_MODEL, NGROUP * KVT], BF16,
                          kind="ExternalInput").ap()
    wq_d = nc.dram_tensor("wq", [D_MODEL, HD], BF16, kind="ExternalInput").ap()
    wk_d = nc.dram_tensor("wk", [D_MODEL, HD], BF16, kind="ExternalInput").ap()
    wv_d = nc.dram_tensor("wv", [D_MODEL, HD], BF16, kind="ExternalInput").ap()
    wo_d = nc.dram_tensor("wo", [HD, D_MODEL], BF16, kind="ExternalInput").ap()
    bq_d = nc.dram_tensor("bq", [HD], F32, kind="ExternalInput").ap()
    bk_d = nc.dram_tensor("bk", [HD], F32, kind="ExternalInput").ap()
    bv_d = nc.dram_tensor("bv", [HD], F32, kind="ExternalInput").ap()
    bo_d = nc.dram_tensor("bo", [D_MODEL], F32, kind="ExternalInput").ap()
    out_d = nc.dram_tensor("out", [TOK, D_MODEL], F32,
                           kind="ExternalOutput").ap()

    h_r = hslab_d.rearrange("(t p) d -> p t d", p=128)     # [128, 8, 1024]
    eT_r = eT_d.rearrange("(k p) t -> p k t", p=128)       # [128, 8, 4096]
    wq_r = wq_d.rearrange("(k p) n -> p k n", p=128)
    wk_r = wk_d.rearrange("(k p) n -> p k n", p=128)
    wv_r = wv_d.rearrange("(k p) n -> p k n", p=128)
    wo_r = wo_d.rearrange("(k p) n -> p k n", p=128)
    bq_r = bq_d.rearrange("(m p) -> p m", p=128)           # [128, 8]
    bk_r = bk_d.rearrange("(m p) -> p m", p=128)
    out_r = out_d.rearrange("(g p) d -> p g d", p=128)     # [128, 8, 1024]

    from contextlib import ExitStack

    with tile.TileContext(nc) as tc, ExitStack() as ctx:
        consts = ctx.enter_context(tc.tile_pool(name="consts", bufs=1))
        p1 = ctx.enter_context(tc.tile_pool(name="p1", bufs=1))
        hn_pool = ctx.enter_context(tc.tile_pool(name="hn", bufs=4))
        stats = ctx.enter_context(tc.tile_pool(name="stats", bufs=6))
        et_pool = ctx.enter_context(tc.tile_pool(name="et", bufs=2))
        kt_pool = ctx.enter_context(tc.tile_pool(name="kt", bufs=2))
        v_pool = ctx.enter_context(tc.tile_pool(name="v", bufs=2))
        exp_pool = ctx.enter_context(tc.tile_pool(name="expp", bufs=2))
        ptr_pool = ctx.enter_context(tc.tile_pool(name="ptr", bufs=1))
        ao_pool = ctx.enter_context(tc.tile_pool(name="ao", bufs=2))
        out_pool = ctx.enter_context(tc.tile_pool(name="outp", bufs=2))
        # Separate PSUM pools per stage: a single pool's FIFO slot queue
        # couples the group-g tail to group-g+1 projections and serializes
        # the whole group loop.
        ps_proj = ctx.enter_context(
            tc.tile_pool(name="ps_proj", bufs=2, space="PSUM"))
        ps_sc = ctx.enter_context(
            tc.tile_pool(name="ps_sc", bufs=2, space="PSUM"))
        ps_tr = ctx.enter_context(
            tc.tile_pool(name="ps_tr", bufs=2, space="PSUM"))
        ps_av = ctx.enter_context(
            tc.tile_pool(name="ps_av", bufs=2, space="PSUM"))

        # ---- constants ----
        # DMA issue order matters for the startup critical path: wk, bk and
        # e^T(g0) go first so the group-0 K projection can start while
        # h/wq/wv are still loading.
        wk_sb = consts.tile([128, KD, HD], BF16, tag="wk")
        bk_sb = consts.tile([128, KD], F32, tag="bk")
        nc.sync.dma_start(out=bk_sb[:], in_=bk_r)
        ident = consts.tile([128, 128], BF16, tag="ident")
        make_identity(nc, ident[:])
        if add_bv:
            bv_rep = consts.tile([128, HD], F32, tag="bvrep")
            nc.gpsimd.dma_start(
                out=bv_rep[:],
                in_=bass.AP(tensor=bv_d.tensor, offset=0,
                            ap=[[0, 128], [1, HD]]))
        if add_bo:
            bo_rep = consts.tile([128, D_MODEL], F32, tag="borep")
            nc.gpsimd.dma_start(
                out=bo_rep[:],
                in_=bass.AP(tensor=bo_d.tensor, offset=0,
                            ap=[[0, 128], [1, D_MODEL]]))

        state = {}

        def emit_k(g):
            """e^T load + K^T projection (feature-major)."""
            et_g = et_pool.tile([128, KD, KVT], BF16, tag="et")
            if g == 0:
                # interleave wk and e^T k-tiles so matmul (m=0, k) can start
                # as soon as pair k has landed
                for k in range(KD):
                    nc.sync.dma_start(out=wk_sb[:, k, :], in_=wk_r[:, k, :])
                    nc.sync.dma_start(
                        out=et_g[:, k, :],
                        in_=eT_r[:, k, g * KVT:(g + 1) * KVT])
            else:
                nc.sync.dma_start(
                    out=et_g[:], in_=eT_r[:, :, g * KVT:(g + 1) * KVT])
            kT = kt_pool.tile([128, KD, KVT], BF16, tag="kt")
            for m in range(KD):
                ps = ps_proj.tile([128, 512], F32, tag="proj")
                for k in range(KD):
                    nc.tensor.matmul(
                        ps[:], wk_sb[:, k, m * 128:(m + 1) * 128],
                        et_g[:, k, :],
                        start=(k == 0), stop=(k == KD - 1))
                nc.scalar.activation(
                    kT[:, m, :], ps[:], AF.Identity,
                    bias=bk_sb[:, m:m + 1])
            state[g] = {"et": et_g, "kT": kT}

        def emit_scores(g, with_v):
            """scores + exp for all heads; head pairs share one psum bank
            ([128, 2, 256]) and one Exp activation. With with_v=True one
            V-projection psum group is interleaved per head pair so the PE
            never stalls on the ACT exp pacing. Row sums via DVE reduce."""
            kT = state[g]["kT"]
            et_g = state[g]["et"]
            if "expb" in state[g]:
                expb = state[g]["expb"]
                sums = state[g]["sums"]
                heads = range(N_HEADS // 2, N_HEADS)
            else:
                expb = exp_pool.tile([128, N_HEADS, 256], BF16, tag="expb")
                sums = stats.tile([128, N_HEADS], F32, tag="sums")
                heads = range(N_HEADS) if not with_v else range(N_HEADS // 2)
            for hp in heads:
                h = hp
                ps = ps_sc.tile([128, 256], F32, tag="sc")
                po = (h % 2) * 64
                for c in range(2):
                    nc.tensor.matmul(
                        ps[c * 64:(c + 1) * 64, :],
                        qT[po:po + 64, h // 2,
                           g * GTOK + c * 64:g * GTOK + (c + 1) * 64],
                        kT[po:po + 64, h // 2, c * 256:(c + 1) * 256],
                        start=True, stop=True)
                nc.scalar.activation(
                    expb[:, h, :], ps[:], AF.Exp, scale=SCALE,
                    accum_out=sums[:, h:h + 1])
            state[g]["expb"] = expb
            state[g]["sums"] = sums

        def emit_v(g):
            """V projection (token-major)."""
            et_g = state[g]["et"]
            v_g = v_pool.tile([128, 4, HD], BF16, tag="v")
            for m in range(4):
                for n in range(2):
                    ps = ps_proj.tile([128, 512], F32, tag="proj")
                    for k in range(KD):
                        nc.tensor.matmul(
                            ps[:], et_g[:, k, m * 128:(m + 1) * 128],
                            wv_sb[:, k, n * 512:(n + 1) * 512],
                            start=(k == 0), stop=(k == KD - 1))
                    if add_bv:
                        nc.vector.tensor_add(
                            v_g[:, m, n * 512:(n + 1) * 512], ps[:],
                            bv_rep[:, n * 512:(n + 1) * 512])
                    else:
                        nc.scalar.activation(
                            v_g[:, m, n * 512:(n + 1) * 512], ps[:], AF.Copy)
            state[g]["v"] = v_g

        def emit_probnorm(g):
            """Normalize exp by row sums (in place, bf16)."""
            sums = state[g]["sums"]
            expb = state[g]["expb"]
            recip = stats.tile([128, N_HEADS], F32, tag="recip")
            nc.vector.reciprocal(recip[:], sums[:])
            recip_b = stats.tile([128, N_HEADS], BF16, tag="recipb")
            nc.vector.tensor_copy(out=recip_b[:], in_=recip[:])
            nc.vector.tensor_mul(
                expb[:], expb[:],
                recip_b[:].to_broadcast([128, N_HEADS, 256]))

        def emit_tail(g):
            st_g = state.pop(g)
            v_g, expb = st_g["v"], st_g["expb"]

            # probs^T via full-width PE transposes: block (h, f) transposes
            # [128 (2c x i), 128 nj] -> [128 nj, 128 (2c x i)]; 8 blocks
            # share one psum bank (bf16), one copy per bank.
            pT = ptr_pool.tile([128, 32, 128], BF16, tag="pT")
            for blk in range(4):
                ps = ps_tr.tile([128, 8, 128], BF16, tag="tr")
                for s in range(8):
                    idx = blk * 8 + s  # idx = h*2 + f
                    h = idx // 2
                    f = idx % 2
                    nc.tensor.transpose(
                        ps[:, s, :],
                        expb[:, h, f * 128:(f + 1) * 128],
                        ident[:])
                nc.vector.tensor_copy(
                    out=pT[:, blk * 8:(blk + 1) * 8, :], in_=ps[:])

            # attnout^T[dv, i] per head; head pairs share a psum tile.
            aout = ao_pool.tile([128, KD, GTOK], BF16, tag="aout")
            for hp in range(KD):
                ps = ps_av.tile([128, 128], F32, tag="av")
                for c in range(2):
                    for dh in range(2):
                        h = hp * 2 + dh
                        for f in range(2):
                            nc.tensor.matmul(
                                ps[dh * 64:(dh + 1) * 64,
                                   c * 64:(c + 1) * 64],
                                v_g[:, c * 2 + f, h * 64:(h + 1) * 64],
                                pT[:, h * 2 + f, c * 64:(c + 1) * 64],
                                start=(f == 0), stop=(f == 1))
                nc.vector.tensor_copy(out=aout[:, hp, :], in_=ps[:])

            # O projection (token-major) + residual
            for n in range(2):
                ps = ps_proj.tile([128, 512], F32, tag="proj")
                for k in range(KD):
                    nc.tensor.matmul(
                        ps[:], aout[:, k, :],
                        wo_sb[:, k, n * 512:(n + 1) * 512],
                        start=(k == 0), stop=(k == KD - 1))
                outb = out_pool.tile([128, 512], F32, tag="outb")
                nc.vector.tensor_add(
                    outb[:], ps[:], hsb[:, g, n * 512:(n + 1) * 512])
                if add_bo:
                    nc.vector.tensor_add(
                        outb[:], outb[:], bo_rep[:, n * 512:(n + 1) * 512])
                nc.sync.dma_start(
                    out=out_r[:, g, n * 512:(n + 1) * 512], in_=outb[:])

        # group-0 K projection first: it only needs wk + e^T(g0), and fills
        # the PE while h loads and layernorm runs on DVE.
        emit_k(0)

        hsb = p1.tile([128, KD, D_MODEL], F32, tag="hslab")
        for tt in range(KD):
            nc.sync.dma_start(out=hsb[:, tt, :], in_=h_r[:, tt, :])
        wq_sb = consts.tile([128, KD, HD], BF16, tag="wqo")
        nc.sync.dma_start(out=wq_sb[:], in_=wq_r)
        wv_sb = consts.tile([128, KD, HD], BF16, tag="wv")
        nc.sync.dma_start(out=wv_sb[:], in_=wv_r)
        bq_sb = consts.tile([128, KD], F32, tag="bq")
        nc.sync.dma_start(out=bq_sb[:], in_=bq_r)

        # ---- phase 1: layernorm -> hn^T -> Q^T ----
        # Batched stats: one Sqrt activation for all 8 row-tiles keeps the
        # ACT sync-wait count within the walrus per-instruction limit.
        mv = stats.tile([128, KD, 2], F32, tag="mv")
        for tt in range(KD):
            st = stats.tile([128, 2, 6], F32, tag="bnst")
            nc.vector.bn_stats(out=st[:, 0, :], in_=hsb[:, tt, 0:512])
            nc.vector.bn_stats(out=st[:, 1, :], in_=hsb[:, tt, 512:1024])
            nc.vector.bn_aggr(out=mv[:, tt, :], in_=st[:])
        var_eps = stats.tile([128, KD], F32, tag="vareps")
        nc.vector.tensor_scalar_add(
            out=var_eps[:], in0=mv[:, :, 1], scalar1=EPS)
        std = stats.tile([128, KD], F32, tag="std")
        nc.scalar.activation(std[:], var_eps[:], AF.Sqrt)
        rstd = stats.tile([128, KD], F32, tag="rstd")
        nc.vector.reciprocal(rstd[:], std[:])

        hnT = p1.tile([128, KD, TOK], BF16, tag="hnT")
        for tb in range(2):  # 512-token halves
            hn_tiles = []
            for q in range(4):
                tt = tb * 4 + q
                hn_t = hn_pool.tile([128, D_MODEL], BF16, tag="hn")
                nc.vector.tensor_scalar(
                    out=hn_t[:], in0=hsb[:, tt, :],
                    scalar1=mv[:, tt, 0:1], scalar2=rstd[:, tt:tt + 1],
                    op0=ALU.subtract, op1=ALU.mult)
                hn_tiles.append(hn_t)
            for dk in range(KD):
                ps = ps_tr.tile([128, 512], BF16, tag="tr")
                for q in range(4):
                    nc.tensor.transpose(
                        ps[:, q * 128:(q + 1) * 128],
                        hn_tiles[q][:, dk * 128:(dk + 1) * 128],
                        ident[:])
                nc.vector.tensor_copy(
                    out=hnT[:, dk, tb * 512:(tb + 1) * 512], in_=ps[:])

        emit_v(0)

        qT = p1.tile([128, KD, TOK], BF16, tag="qT")
        for m in range(KD):
            for n in range(2):
                ps = ps_proj.tile([128, 512], F32, tag="proj")
                for k in range(KD):
                    nc.tensor.matmul(
                        ps[:], wq_sb[:, k, m * 128:(m + 1) * 128],
                        hnT[:, k, n * 512:(n + 1) * 512],
                        start=(k == 0), stop=(k == KD - 1))
                nc.scalar.activation(
                    qT[:, m, n * 512:(n + 1) * 512], ps[:], AF.Identity,
                    bias=bq_sb[:, m:m + 1])

        # wo reuses wq's slot; DMA waits for the last wq read.
        wo_sb = consts.tile([128, KD, D_MODEL], BF16, tag="wqo")
        nc.sync.dma_start(out=wo_sb[:], in_=wo_r)

        emit_scores(0, with_v=False)
        for g in range(1, NGROUP):
            emit_k(g)
            emit_probnorm(g - 1)
            emit_scores(g, with_v=True)   # heads 0-7
            emit_v(g)
            emit_scores(g, with_v=True)   # heads 8-15
            emit_tail(g - 1)
        emit_probnorm(NGROUP - 1)
        emit_tail(NGROUP - 1)

    nc.compile()
    return nc


_PROG_CACHE: dict = {}


def _get_program(add_bv: bool, add_bo: bool) -> bass.Bass:
    key = (add_bv, add_bo)
    if key not in _PROG_CACHE:
        _PROG_CACHE[key] = _build(add_bv, add_bo)
    return _PROG_CACHE[key]


def make_in_maps(h, e, Wq, bq, Wk, bk, Wv, bv, Wo, bo, gamma, beta):
    """Host-side sharding/layout prep. Returns (in_maps, add_bv, add_bo)."""
    h = np.asarray(h, dtype=np.float32)
    e = np.asarray(e, dtype=np.float32)
    Wq = np.asarray(Wq, dtype=np.float32)
    bq = np.asarray(bq, dtype=np.float32)
    Wk = np.asarray(Wk, dtype=np.float32)
    bk = np.asarray(bk, dtype=np.float32)
    Wv = np.asarray(Wv, dtype=np.float32)
    bv = np.asarray(bv, dtype=np.float32)
    Wo = np.asarray(Wo, dtype=np.float32)
    bo = np.asarray(bo, dtype=np.float32)
    gamma = np.asarray(gamma, dtype=np.float32)
    beta = np.asarray(beta, dtype=np.float32)

    # fold layernorm affine into the Q projection
    Wq_f = gamma[:, None] * Wq
    bq_f = bq + beta @ Wq

    wq_b = np.ascontiguousarray(Wq_f.astype(NP_BF16))
    wk_b = np.ascontiguousarray(Wk.astype(NP_BF16))
    wv_b = np.ascontiguousarray(Wv.astype(NP_BF16))
    wo_b = np.ascontiguousarray(Wo.astype(NP_BF16))

    add_bv = bool(np.any(bv != 0.0))
    add_bo = bool(np.any(bo != 0.0))

    in_maps = []
    for core in range(N_CORES):
        b = core // 2
        c0 = (core % 2) * CH_PER_CORE
        t0 = c0 * CHUNK_LEN + CHUNK_LEN - 1
        hslab = np.zeros((TOK, D_MODEL), np.float32)
        valid = min(TOK, SEQ - t0)
        hslab[:valid] = h[b, t0:t0 + valid]
        eT = np.ascontiguousarray(
            e[b, c0:c0 + CH_PER_CORE].reshape(NGROUP * KVT, D_MODEL)
            .T.astype(NP_BF16))
        in_maps.append({
            "hslab": hslab,
            "eT": eT,
            "wq": wq_b, "wk": wk_b, "wv": wv_b, "wo": wo_b,
            "bq": bq_f.astype(np.float32), "bk": bk,
            "bv": bv, "bo": bo,
        })
    return in_maps, add_bv, add_bo


def assemble(h, core_outs):
    """Gather per-core [TOK, D_MODEL] slabs into the full output."""
    h = np.asarray(h, dtype=np.float32)
    out = h.copy()
    for core, res in enumerate(core_outs):
        b = core // 2
        c0 = (core % 2) * CH_PER_CORE
        t0 = c0 * CHUNK_LEN + CHUNK_LEN - 1
        valid = min(TOK, SEQ - t0)
        out[b, t0:t0 + valid] = res[:valid]
    return out


def kernel(h, e, Wq, bq, Wk, bk, Wv, bv, Wo, bo, gamma, beta):
    from concourse.bass_utils import run_bass_kernel_spmd

    in_maps, add_bv, add_bo = make_in_maps(
        h, e, Wq, bq, Wk, bk, Wv, bv, Wo, bo, gamma, beta)
    nc = _get_program(add_bv, add_bo)
    res = run_bass_kernel_spmd(nc, in_maps, list(range(N_CORES)))
    core_outs = [r["out"] for r in res.results]
    return assemble(h, core_outs)


# revision 28
# speedup vs baseline: 1.2215x; 1.2215x over previous
"""Chunked cross-attention (RETRO-style) TRN2 Bass kernel.

Sharding: pure data-parallel over (batch, chunk-block): 8 cores =
4 batches x 2 chunk-halves (16 chunks each). Every (batch, chunk) pair
attends only to its own retrieved neighbors, and all projections are
row-wise, so there is no cross-core communication at all.

Per-core pipeline (all matmul operands bf16, fp32 PSUM accumulation,
fp32 layernorm/softmax statistics):
  phase 1: LN(h rows) -> PE-transpose -> Q^T = Wq^T @ hn^T  (feature-major)
  per chunk-pair group (8 groups x 2 chunks):
    K^T = Wk^T @ e^T   (feature-major, e^T pre-transposed on host)
    V   = e @ Wv       (token-major, e^T as stationary operand)
    scores[c,h] = Q_h^T.T @ K_h^T  (both chunks packed into 128 psum
                  partitions via PE quadrant placement)
    exp + row-sum fused on ACT (accum_out); probs = exp * recip(sum)
    probs^T via PE transpose; attnout = V.T @ probs^T (feature-major)
    out2 = attnout.T @ Wo (token-major) + h residual -> DMA out
gamma/beta are folded into Wq/bq on the host; bq/bk are fused into the
PSUM->SBUF copies; bv/bo paths are emitted only if nonzero.
"""

import os
import sys

import numpy as np

for _p in ("/opt/trn_rl_repo", "/root/.axon_site/_ro/trn_rl_repo"):
    if os.path.isdir(_p) and _p not in sys.path:
        sys.path.append(_p)

import ml_dtypes  # noqa: E402

import concourse.bass as bass  # noqa: E402
import concourse.bacc as bacc  # noqa: E402
import concourse.mybir as mybir  # noqa: E402
import concourse.tile as tile  # noqa: E402
from concourse.masks import make_identity  # noqa: E402

BF16 = mybir.dt.bfloat16
F32 = mybir.dt.float32
AF = mybir.ActivationFunctionType
ALU = mybir.AluOpType
NP_BF16 = ml_dtypes.bfloat16

D_MODEL = 1024
N_HEADS = 16
D_K = 64
HD = N_HEADS * D_K  # 1024
CHUNK_LEN = 64
SEQ = 2048
BATCH = 4
N_CORES = 8
CH_PER_CORE = 16
TOK = CH_PER_CORE * CHUNK_LEN  # 1024 q-side tokens per core
NGROUP = 8                     # chunk-pair groups per core
GTOK = 128                     # q tokens per group (2 chunks)
KVT = 512                      # kv tokens per group (2 chunks * 2 * 128)
KD = D_MODEL // 128            # 8 contraction tiles
EPS = 1e-5
SCALE = 1.0 / float(np.sqrt(D_K))


def _build(add_bv: bool, add_bo: bool) -> bass.Bass:
    nc = bacc.Bacc("TRN2", target_bir_lowering=False, debug=False,
                   num_devices=N_CORES)

    hslab_d = nc.dram_tensor("hslab", [TOK, D_MODEL], F32,
                             kind="ExternalInput").ap()
    eT_d = nc.dram_tensor("eT", [D_MODEL, NGROUP * KVT], BF16,
                          kind="ExternalInput").ap()
    wq_d = nc.dram_tensor("wq", [D_MODEL, HD], BF16, kind="ExternalInput").ap()
    wk_d = nc.dram_tensor("wk", [D_MODEL, HD], BF16, kind="ExternalInput").ap()
    wv_d = nc.dram_tensor("wv", [D_MODEL, HD], BF16, kind="ExternalInput").ap()
    wo_d = nc.dram_tensor("wo", [HD, D_MODEL], BF16, kind="ExternalInput").ap()
    bq_d = nc.dram_tensor("bq", [HD], F32, kind="ExternalInput").ap()
    bk_d = nc.dram_tensor("bk", [HD], F32, kind="ExternalInput").ap()
    bv_d = nc.dram_tensor("bv", [HD], F32, kind="ExternalInput").ap()
    bo_d = nc.dram_tensor("bo", [D_MODEL], F32, kind="ExternalInput").ap()
    out_d = nc.dram_tensor("out", [TOK, D_MODEL], F32,
                           kind="ExternalOutput").ap()

    h_r = hslab_d.rearrange("(t p) d -> p t d", p=128)     # [128, 8, 1024]
    eT_r = eT_d.rearrange("(k p) t -> p k t", p=128)       # [128, 8, 4096]
    wq_r = wq_d.rearrange("(k p) n -> p k n", p=128)
    wk_r = wk_d.rearrange("(k p) n -> p k n", p=128)
    wv_r = wv_d.rearrange("(k p) n -> p k n", p=128)
    wo_r = wo_d.rearrange("(k p) n -> p k n", p=128)
    bq_r = bq_d.rearrange("(m p) -> p m", p=128)           # [128, 8]
    bk_r = bk_d.rearrange("(m p) -> p m", p=128)
    out_r = out_d.rearrange("(g p) d -> p g d", p=128)     # [128, 8, 1024]

    from contextlib import ExitStack

    with tile.TileContext(nc) as tc, ExitStack() as ctx:
        consts = ctx.enter_context(tc.tile_pool(name="consts", bufs=1))
        p1 = ctx.enter_context(tc.tile_pool(name="p1", bufs=1))
        hn_pool = ctx.enter_context(tc.tile_pool(name="hn", bufs=4))
        stats = ctx.enter_context(tc.tile_pool(name="stats", bufs=6))
        et_pool = ctx.enter_context(tc.tile_pool(name="et", bufs=2))
        kt_pool = ctx.enter_context(tc.tile_pool(name="kt", bufs=2))
        v_pool = ctx.enter_context(tc.tile_pool(name="v", bufs=2))
        exp_pool = ctx.enter_context(tc.tile_pool(name="expp", bufs=2))
        ptr_pool = ctx.enter_context(tc.tile_pool(name="ptr", bufs=1))
        ao_pool = ctx.enter_context(tc.tile_pool(name="ao", bufs=2))
        out_pool = ctx.enter_context(tc.tile_pool(name="outp", bufs=2))
        # Separate PSUM pools per stage: a single pool's FIFO slot queue
        # couples the group-g tail to group-g+1 projections and serializes
        # the whole group loop.
        ps_proj = ctx.enter_context(
            tc.tile_pool(name="ps_proj", bufs=2, space="PSUM"))
        ps_sc = ctx.enter_context(
            tc.tile_pool(name="ps_sc", bufs=2, space="PSUM"))
        ps_tr = ctx.enter_context(
            tc.tile_pool(name="ps_tr", bufs=2, space="PSUM"))
        ps_av = ctx.enter_context(
            tc.tile_pool(name="ps_av", bufs=2, space="PSUM"))

        # ---- constants ----
        # DMA issue order matters for the startup critical path: wk, bk and
        # e^T(g0) go first so the group-0 K projection can start while
        # h/wq/wv are still loading.
        wk_sb = consts.tile([128, KD, HD], BF16, tag="wk")
        bk_sb = consts.tile([128, KD], F32, tag="bk")
        nc.sync.dma_start(out=bk_sb[:], in_=bk_r)
        ident = consts.tile([128, 128], BF16, tag="ident")
        make_identity(nc, ident[:])
        if add_bv:
            bv_rep = consts.tile([128, HD], F32, tag="bvrep")
            nc.gpsimd.dma_start(
                out=bv_rep[:],
                in_=bass.AP(tensor=bv_d.tensor, offset=0,
                            ap=[[0, 128], [1, HD]]))
        if add_bo:
            bo_rep = consts.tile([128, D_MODEL], F32, tag="borep")
            nc.gpsimd.dma_start(
                out=bo_rep[:],
                in_=bass.AP(tensor=bo_d.tensor, offset=0,
                            ap=[[0, 128], [1, D_MODEL]]))

        state = {}

        def emit_k(g):
            """e^T load + K^T projection (feature-major)."""
            et_g = et_pool.tile([128, KD, KVT], BF16, tag="et")
            if g == 0:
                # interleave wk and e^T k-tiles so matmul (m=0, k) can start
                # as soon as pair k has landed
                for k in range(KD):
                    nc.sync.dma_start(out=wk_sb[:, k, :], in_=wk_r[:, k, :])
                    nc.sync.dma_start(
                        out=et_g[:, k, :],
                        in_=eT_r[:, k, g * KVT:(g + 1) * KVT])
            else:
                nc.sync.dma_start(
                    out=et_g[:], in_=eT_r[:, :, g * KVT:(g + 1) * KVT])
            kT = kt_pool.tile([128, KD, KVT], BF16, tag="kt")
            for m in range(KD):
                ps = ps_proj.tile([128, 512], F32, tag="proj")
                for k in range(KD):
                    nc.tensor.matmul(
                        ps[:], wk_sb[:, k, m * 128:(m + 1) * 128],
                        et_g[:, k, :],
                        start=(k == 0), stop=(k == KD - 1))
                nc.scalar.activation(
                    kT[:, m, :], ps[:], AF.Identity,
                    bias=bk_sb[:, m:m + 1])
            state[g] = {"et": et_g, "kT": kT}

        def emit_scores(g, with_v):
            """scores + exp for all heads; head pairs share one psum bank
            ([128, 2, 256]) and one Exp activation. With with_v=True one
            V-projection psum group is interleaved per head pair so the PE
            never stalls on the ACT exp pacing. Row sums via DVE reduce."""
            kT = state[g]["kT"]
            et_g = state[g]["et"]
            if "expb" in state[g]:
                expb = state[g]["expb"]
                sums = state[g]["sums"]
                heads = range(N_HEADS // 2, N_HEADS)
            else:
                expb = exp_pool.tile([128, N_HEADS, 256], BF16, tag="expb")
                sums = stats.tile([128, N_HEADS], F32, tag="sums")
                heads = range(N_HEADS) if not with_v else range(N_HEADS // 2)
            for hp in heads:
                h = hp
                ps = ps_sc.tile([128, 256], F32, tag="sc")
                po = (h % 2) * 64
                for c in range(2):
                    nc.tensor.matmul(
                        ps[c * 64:(c + 1) * 64, :],
                        qT[po:po + 64, h // 2,
                           g * GTOK + c * 64:g * GTOK + (c + 1) * 64],
                        kT[po:po + 64, h // 2, c * 256:(c + 1) * 256],
                        start=True, stop=True)
                nc.scalar.activation(
                    expb[:, h, :], ps[:], AF.Exp, scale=SCALE,
                    accum_out=sums[:, h:h + 1])
            state[g]["expb"] = expb
            state[g]["sums"] = sums

        def emit_v(g):
            """V projection (token-major)."""
            et_g = state[g]["et"]
            v_g = v_pool.tile([128, 4, HD], BF16, tag="v")
            for m in range(4):
                for n in range(2):
                    ps = ps_proj.tile([128, 512], F32, tag="proj")
                    for k in range(KD):
                        nc.tensor.matmul(
                            ps[:], et_g[:, k, m * 128:(m + 1) * 128],
                            wv_sb[:, k, n * 512:(n + 1) * 512],
                            start=(k == 0), stop=(k == KD - 1))
                    if add_bv:
                        nc.vector.tensor_add(
                            v_g[:, m, n * 512:(n + 1) * 512], ps[:],
                            bv_rep[:, n * 512:(n + 1) * 512])
                    else:
                        nc.scalar.activation(
                            v_g[:, m, n * 512:(n + 1) * 512], ps[:], AF.Copy)
            state[g]["v"] = v_g

        def emit_probnorm(g):
            """Normalize exp by row sums (in place, bf16)."""
            sums = state[g]["sums"]
            expb = state[g]["expb"]
            recip = stats.tile([128, N_HEADS], F32, tag="recip")
            nc.vector.reciprocal(recip[:], sums[:])
            recip_b = stats.tile([128, N_HEADS], BF16, tag="recipb")
            nc.vector.tensor_copy(out=recip_b[:], in_=recip[:])
            nc.vector.tensor_mul(
                expb[:], expb[:],
                recip_b[:].to_broadcast([128, N_HEADS, 256]))

        def emit_tail(g):
            st_g = state.pop(g)
            v_g, expb = st_g["v"], st_g["expb"]

            # probs^T via full-width PE transposes: block (h, f) transposes
            # [128 (2c x i), 128 nj] -> [128 nj, 128 (2c x i)]; 8 blocks
            # share one psum bank (bf16), one copy per bank.
            pT = ptr_pool.tile([128, 32, 128], BF16, tag="pT")
            for blk in range(4):
                ps = ps_tr.tile([128, 8, 128], BF16, tag="tr")
                for s in range(8):
                    idx = blk * 8 + s  # idx = h*2 + f
                    h = idx // 2
                    f = idx % 2
                    nc.tensor.transpose(
                        ps[:, s, :],
                        expb[:, h, f * 128:(f + 1) * 128],
                        ident[:])
                nc.vector.tensor_copy(
                    out=pT[:, blk * 8:(blk + 1) * 8, :], in_=ps[:])

            # attnout^T[dv, i] per head; head pairs share a psum tile.
            aout = ao_pool.tile([128, KD, GTOK], BF16, tag="aout")
            for hp in range(KD):
                ps = ps_av.tile([128, 128], F32, tag="av")
                for c in range(2):
                    for dh in range(2):
                        h = hp * 2 + dh
                        for f in range(2):
                            nc.tensor.matmul(
                                ps[dh * 64:(dh + 1) * 64,
                                   c * 64:(c + 1) * 64],
                                v_g[:, c * 2 + f, h * 64:(h + 1) * 64],
                                pT[:, h * 2 + f, c * 64:(c + 1) * 64],
                                start=(f == 0), stop=(f == 1))
                nc.vector.tensor_copy(out=aout[:, hp, :], in_=ps[:])

            # O projection (token-major) + residual
            for n in range(2):
                ps = ps_proj.tile([128, 512], F32, tag="proj")
                for k in range(KD):
                    nc.tensor.matmul(
                        ps[:], aout[:, k, :],
                        wo_sb[:, k, n * 512:(n + 1) * 512],
                        start=(k == 0), stop=(k == KD - 1))
                outb = out_pool.tile([128, 512], F32, tag="outb")
                nc.vector.tensor_add(
                    outb[:], ps[:], hsb[:, g, n * 512:(n + 1) * 512])
                if add_bo:
                    nc.vector.tensor_add(
                        outb[:], outb[:], bo_rep[:, n * 512:(n + 1) * 512])
                nc.sync.dma_start(
                    out=out_r[:, g, n * 512:(n + 1) * 512], in_=outb[:])

        # group-0 K projection first: it only needs wk + e^T(g0), and fills
        # the PE while h loads and layernorm runs on DVE.
        emit_k(0)

        hsb = p1.tile([128, KD, D_MODEL], F32, tag="hslab")
        for tt in range(KD):
            nc.sync.dma_start(out=hsb[:, tt, :], in_=h_r[:, tt, :])
        wq_sb = consts.tile([128, KD, HD], BF16, tag="wqo")
        nc.sync.dma_start(out=wq_sb[:], in_=wq_r)
        wv_sb = consts.tile([128, KD, HD], BF16, tag="wv")
        nc.sync.dma_start(out=wv_sb[:], in_=wv_r)
        bq_sb = consts.tile([128, KD], F32, tag="bq")
        nc.sync.dma_start(out=bq_sb[:], in_=bq_r)

        # ---- phase 1: layernorm -> hn^T -> Q^T ----
        # Batched stats: one Sqrt activation for all 8 row-tiles keeps the
        # ACT sync-wait count within the walrus per-instruction limit.
        mv = stats.tile([128, KD, 2], F32, tag="mv")
        for tt in range(KD):
            st = stats.tile([128, 2, 6], F32, tag="bnst")
            nc.vector.bn_stats(out=st[:, 0, :], in_=hsb[:, tt, 0:512])
            nc.vector.bn_stats(out=st[:, 1, :], in_=hsb[:, tt, 512:1024])
            nc.vector.bn_aggr(out=mv[:, tt, :], in_=st[:])
        var_eps = stats.tile([128, KD], F32, tag="vareps")
        nc.vector.tensor_scalar_add(
            out=var_eps[:], in0=mv[:, :, 1], scalar1=EPS)
        std = stats.tile([128, KD], F32, tag="std")
        nc.scalar.activation(std[:], var_eps[:], AF.Sqrt)
        rstd = stats.tile([128, KD], F32, tag="rstd")
        nc.vector.reciprocal(rstd[:], std[:])

        hnT = p1.tile([128, KD, TOK], BF16, tag="hnT")
        for tb in range(2):  # 512-token halves
            hn_tiles = []
            for q in range(4):
                tt = tb * 4 + q
                hn_t = hn_pool.tile([128, D_MODEL], BF16, tag="hn")
                nc.vector.tensor_scalar(
                    out=hn_t[:], in0=hsb[:, tt, :],
                    scalar1=mv[:, tt, 0:1], scalar2=rstd[:, tt:tt + 1],
                    op0=ALU.subtract, op1=ALU.mult)
                hn_tiles.append(hn_t)
            for dk in range(KD):
                ps = ps_tr.tile([128, 512], BF16, tag="tr")
                for q in range(4):
                    nc.tensor.transpose(
                        ps[:, q * 128:(q + 1) * 128],
                        hn_tiles[q][:, dk * 128:(dk + 1) * 128],
                        ident[:])
                nc.vector.tensor_copy(
                    out=hnT[:, dk, tb * 512:(tb + 1) * 512], in_=ps[:])

        emit_v(0)

        qT = p1.tile([128, KD, TOK], BF16, tag="qT")
        for m in range(KD):
            for n in range(2):
                ps = ps_proj.tile([128, 512], F32, tag="proj")
                for k in range(KD):
                    nc.tensor.matmul(
                        ps[:], wq_sb[:, k, m * 128:(m + 1) * 128],
                        hnT[:, k, n * 512:(n + 1) * 512],
                        start=(k == 0), stop=(k == KD - 1))
                nc.scalar.activation(
                    qT[:, m, n * 512:(n + 1) * 512], ps[:], AF.Identity,
                    bias=bq_sb[:, m:m + 1])

        # wo reuses wq's slot; DMA waits for the last wq read.
        wo_sb = consts.tile([128, KD, D_MODEL], BF16, tag="wqo")
        nc.sync.dma_start(out=wo_sb[:], in_=wo_r)

        emit_scores(0, with_v=False)
        for g in range(1, NGROUP):
            emit_k(g)
            emit_probnorm(g - 1)
            emit_scores(g, with_v=False)
            emit_v(g)
            emit_tail(g - 1)
        emit_probnorm(NGROUP - 1)
        emit_tail(NGROUP - 1)

    nc.compile()
    return nc


_PROG_CACHE: dict = {}


def _get_program(add_bv: bool, add_bo: bool) -> bass.Bass:
    key = (add_bv, add_bo)
    if key not in _PROG_CACHE:
        _PROG_CACHE[key] = _build(add_bv, add_bo)
    return _PROG_CACHE[key]


def make_in_maps(h, e, Wq, bq, Wk, bk, Wv, bv, Wo, bo, gamma, beta):
    """Host-side sharding/layout prep. Returns (in_maps, add_bv, add_bo)."""
    h = np.asarray(h, dtype=np.float32)
    e = np.asarray(e, dtype=np.float32)
    Wq = np.asarray(Wq, dtype=np.float32)
    bq = np.asarray(bq, dtype=np.float32)
    Wk = np.asarray(Wk, dtype=np.float32)
    bk = np.asarray(bk, dtype=np.float32)
    Wv = np.asarray(Wv, dtype=np.float32)
    bv = np.asarray(bv, dtype=np.float32)
    Wo = np.asarray(Wo, dtype=np.float32)
    bo = np.asarray(bo, dtype=np.float32)
    gamma = np.asarray(gamma, dtype=np.float32)
    beta = np.asarray(beta, dtype=np.float32)

    # fold layernorm affine into the Q projection
    Wq_f = gamma[:, None] * Wq
    bq_f = bq + beta @ Wq

    wq_b = np.ascontiguousarray(Wq_f.astype(NP_BF16))
    wk_b = np.ascontiguousarray(Wk.astype(NP_BF16))
    wv_b = np.ascontiguousarray(Wv.astype(NP_BF16))
    wo_b = np.ascontiguousarray(Wo.astype(NP_BF16))

    add_bv = bool(np.any(bv != 0.0))
    add_bo = bool(np.any(bo != 0.0))

    in_maps = []
    for core in range(N_CORES):
        b = core // 2
        c0 = (core % 2) * CH_PER_CORE
        t0 = c0 * CHUNK_LEN + CHUNK_LEN - 1
        hslab = np.zeros((TOK, D_MODEL), np.float32)
        valid = min(TOK, SEQ - t0)
        hslab[:valid] = h[b, t0:t0 + valid]
        eT = np.ascontiguousarray(
            e[b, c0:c0 + CH_PER_CORE].reshape(NGROUP * KVT, D_MODEL)
            .T.astype(NP_BF16))
        in_maps.append({
            "hslab": hslab,
            "eT": eT,
            "wq": wq_b, "wk": wk_b, "wv": wv_b, "wo": wo_b,
            "bq": bq_f.astype(np.float32), "bk": bk,
            "bv": bv, "bo": bo,
        })
    return in_maps, add_bv, add_bo


def assemble(h, core_outs):
    """Gather per-core [TOK, D_MODEL] slabs into the full output."""
    h = np.asarray(h, dtype=np.float32)
    out = h.copy()
    for core, res in enumerate(core_outs):
        b = core // 2
        c0 = (core % 2) * CH_PER_CORE
        t0 = c0 * CHUNK_LEN + CHUNK_LEN - 1
        valid = min(TOK, SEQ - t0)
        out[b, t0:t0 + valid] = res[:valid]
    return out


def kernel(h, e, Wq, bq, Wk, bk, Wv, bv, Wo, bo, gamma, beta):
    from concourse.bass_utils import run_bass_kernel_spmd

    in_maps, add_bv, add_bo = make_in_maps(
        h, e, Wq, bq, Wk, bk, Wv, bv, Wo, bo, gamma, beta)
    nc = _get_program(add_bv, add_bo)
    res = run_bass_kernel_spmd(nc, in_maps, list(range(N_CORES)))
    core_outs = [r["out"] for r in res.results]
    return assemble(h, core_outs)


# revision 29
# speedup vs baseline: 1.2402x; 1.0154x over previous
"""Chunked cross-attention (RETRO-style) TRN2 Bass kernel.

Sharding: pure data-parallel over (batch, chunk-block): 8 cores =
4 batches x 2 chunk-halves (16 chunks each). Every (batch, chunk) pair
attends only to its own retrieved neighbors, and all projections are
row-wise, so there is no cross-core communication at all.

Per-core pipeline (all matmul operands bf16, fp32 PSUM accumulation,
fp32 layernorm/softmax statistics):
  phase 1: LN(h rows) -> PE-transpose -> Q^T = Wq^T @ hn^T  (feature-major)
  per chunk-pair group (8 groups x 2 chunks):
    K^T = Wk^T @ e^T   (feature-major, e^T pre-transposed on host)
    V   = e @ Wv       (token-major, e^T as stationary operand)
    scores[c,h] = Q_h^T.T @ K_h^T  (both chunks packed into 128 psum
                  partitions via PE quadrant placement)
    exp + row-sum fused on ACT (accum_out); probs = exp * recip(sum)
    probs^T via PE transpose; attnout = V.T @ probs^T (feature-major)
    out2 = attnout.T @ Wo (token-major) + h residual -> DMA out
gamma/beta are folded into Wq/bq on the host; bq/bk are fused into the
PSUM->SBUF copies; bv/bo paths are emitted only if nonzero.
"""

import os
import sys

import numpy as np

for _p in ("/opt/trn_rl_repo", "/root/.axon_site/_ro/trn_rl_repo"):
    if os.path.isdir(_p) and _p not in sys.path:
        sys.path.append(_p)

import ml_dtypes  # noqa: E402

import concourse.bass as bass  # noqa: E402
import concourse.bacc as bacc  # noqa: E402
import concourse.mybir as mybir  # noqa: E402
import concourse.tile as tile  # noqa: E402
from concourse.masks import make_identity  # noqa: E402

BF16 = mybir.dt.bfloat16
F32 = mybir.dt.float32
AF = mybir.ActivationFunctionType
ALU = mybir.AluOpType
NP_BF16 = ml_dtypes.bfloat16

D_MODEL = 1024
N_HEADS = 16
D_K = 64
HD = N_HEADS * D_K  # 1024
CHUNK_LEN = 64
SEQ = 2048
BATCH = 4
N_CORES = 8
CH_PER_CORE = 16
TOK = CH_PER_CORE * CHUNK_LEN  # 1024 q-side tokens per core
NGROUP = 8                     # chunk-pair groups per core
GTOK = 128                     # q tokens per group (2 chunks)
KVT = 512                      # kv tokens per group (2 chunks * 2 * 128)
KD = D_MODEL // 128            # 8 contraction tiles
EPS = 1e-5
SCALE = 1.0 / float(np.sqrt(D_K))


def _build(add_bv: bool, add_bo: bool) -> bass.Bass:
    nc = bacc.Bacc("TRN2", target_bir_lowering=False, debug=False,
                   num_devices=N_CORES)

    hslab_d = nc.dram_tensor("hslab", [TOK, D_MODEL], F32,
                             kind="ExternalInput").ap()
    eT_d = nc.dram_tensor("eT", [D_MODEL, NGROUP * KVT], BF16,
                          kind="ExternalInput").ap()
    wq_d = nc.dram_tensor("wq", [D_MODEL, HD], BF16, kind="ExternalInput").ap()
    wk_d = nc.dram_tensor("wk", [D_MODEL, HD], BF16, kind="ExternalInput").ap()
    wv_d = nc.dram_tensor("wv", [D_MODEL, HD], BF16, kind="ExternalInput").ap()
    wo_d = nc.dram_tensor("wo", [HD, D_MODEL], BF16, kind="ExternalInput").ap()
    bq_d = nc.dram_tensor("bq", [HD], F32, kind="ExternalInput").ap()
    bk_d = nc.dram_tensor("bk", [HD], F32, kind="ExternalInput").ap()
    bv_d = nc.dram_tensor("bv", [HD], F32, kind="ExternalInput").ap()
    bo_d = nc.dram_tensor("bo", [D_MODEL], F32, kind="ExternalInput").ap()
    out_d = nc.dram_tensor("out", [TOK, D_MODEL], F32,
                           kind="ExternalOutput").ap()

    h_r = hslab_d.rearrange("(t p) d -> p t d", p=128)     # [128, 8, 1024]
    eT_r = eT_d.rearrange("(k p) t -> p k t", p=128)       # [128, 8, 4096]
    wq_r = wq_d.rearrange("(k p) n -> p k n", p=128)
    wk_r = wk_d.rearrange("(k p) n -> p k n", p=128)
    wv_r = wv_d.rearrange("(k p) n -> p k n", p=128)
    wo_r = wo_d.rearrange("(k p) n -> p k n", p=128)
    bq_r = bq_d.rearrange("(m p) -> p m", p=128)           # [128, 8]
    bk_r = bk_d.rearrange("(m p) -> p m", p=128)
    out_r = out_d.rearrange("(g p) d -> p g d", p=128)     # [128, 8, 1024]

    from contextlib import ExitStack

    with tile.TileContext(nc) as tc, ExitStack() as ctx:
        consts = ctx.enter_context(tc.tile_pool(name="consts", bufs=1))
        p1 = ctx.enter_context(tc.tile_pool(name="p1", bufs=1))
        hn_pool = ctx.enter_context(tc.tile_pool(name="hn", bufs=4))
        stats = ctx.enter_context(tc.tile_pool(name="stats", bufs=6))
        et_pool = ctx.enter_context(tc.tile_pool(name="et", bufs=2))
        kt_pool = ctx.enter_context(tc.tile_pool(name="kt", bufs=2))
        v_pool = ctx.enter_context(tc.tile_pool(name="v", bufs=2))
        exp_pool = ctx.enter_context(tc.tile_pool(name="expp", bufs=2))
        ptr_pool = ctx.enter_context(tc.tile_pool(name="ptr", bufs=1))
        ao_pool = ctx.enter_context(tc.tile_pool(name="ao", bufs=2))
        out_pool = ctx.enter_context(tc.tile_pool(name="outp", bufs=2))
        # Separate PSUM pools per stage: a single pool's FIFO slot queue
        # couples the group-g tail to group-g+1 projections and serializes
        # the whole group loop.
        ps_proj = ctx.enter_context(
            tc.tile_pool(name="ps_proj", bufs=2, space="PSUM"))
        ps_sc = ctx.enter_context(
            tc.tile_pool(name="ps_sc", bufs=2, space="PSUM"))
        ps_tr = ctx.enter_context(
            tc.tile_pool(name="ps_tr", bufs=2, space="PSUM"))
        ps_av = ctx.enter_context(
            tc.tile_pool(name="ps_av", bufs=2, space="PSUM"))

        # ---- constants ----
        # DMA issue order matters for the startup critical path: wk, bk and
        # e^T(g0) go first so the group-0 K projection can start while
        # h/wq/wv are still loading.
        wk_sb = consts.tile([128, KD, HD], BF16, tag="wk")
        bk_sb = consts.tile([128, KD], F32, tag="bk")
        nc.sync.dma_start(out=bk_sb[:], in_=bk_r)
        ident = consts.tile([128, 128], BF16, tag="ident")
        make_identity(nc, ident[:])
        if add_bv:
            bv_rep = consts.tile([128, HD], F32, tag="bvrep")
            nc.gpsimd.dma_start(
                out=bv_rep[:],
                in_=bass.AP(tensor=bv_d.tensor, offset=0,
                            ap=[[0, 128], [1, HD]]))
        if add_bo:
            bo_rep = consts.tile([128, D_MODEL], F32, tag="borep")
            nc.gpsimd.dma_start(
                out=bo_rep[:],
                in_=bass.AP(tensor=bo_d.tensor, offset=0,
                            ap=[[0, 128], [1, D_MODEL]]))

        state = {}

        def emit_k(g):
            """e^T load + K^T projection (feature-major)."""
            et_g = et_pool.tile([128, KD, KVT], BF16, tag="et")
            if g == 0:
                # interleave wk and e^T half-slabs: 4 big DMAs instead of
                # 16 small ones (each small DMA pays ~1us queue overhead,
                # which dominated the startup critical path)
                for kb in range(2):
                    ks = slice(kb * 4, (kb + 1) * 4)
                    nc.sync.dma_start(out=wk_sb[:, ks, :], in_=wk_r[:, ks, :])
                    nc.sync.dma_start(
                        out=et_g[:, ks, :],
                        in_=eT_r[:, ks, g * KVT:(g + 1) * KVT])
            else:
                nc.sync.dma_start(
                    out=et_g[:], in_=eT_r[:, :, g * KVT:(g + 1) * KVT])
            kT = kt_pool.tile([128, KD, KVT], BF16, tag="kt")
            for m in range(KD):
                ps = ps_proj.tile([128, 512], F32, tag="proj")
                for k in range(KD):
                    nc.tensor.matmul(
                        ps[:], wk_sb[:, k, m * 128:(m + 1) * 128],
                        et_g[:, k, :],
                        start=(k == 0), stop=(k == KD - 1))
                nc.scalar.activation(
                    kT[:, m, :], ps[:], AF.Identity,
                    bias=bk_sb[:, m:m + 1])
            state[g] = {"et": et_g, "kT": kT}

        def emit_scores(g, with_v):
            """scores + exp for all heads; head pairs share one psum bank
            ([128, 2, 256]) and one Exp activation. With with_v=True one
            V-projection psum group is interleaved per head pair so the PE
            never stalls on the ACT exp pacing. Row sums via DVE reduce."""
            kT = state[g]["kT"]
            et_g = state[g]["et"]
            if "expb" in state[g]:
                expb = state[g]["expb"]
                sums = state[g]["sums"]
                heads = range(N_HEADS // 2, N_HEADS)
            else:
                expb = exp_pool.tile([128, N_HEADS, 256], BF16, tag="expb")
                sums = stats.tile([128, N_HEADS], F32, tag="sums")
                heads = range(N_HEADS) if not with_v else range(N_HEADS // 2)
            for hp in heads:
                h = hp
                ps = ps_sc.tile([128, 256], F32, tag="sc")
                po = (h % 2) * 64
                for c in range(2):
                    nc.tensor.matmul(
                        ps[c * 64:(c + 1) * 64, :],
                        qT[po:po + 64, h // 2,
                           g * GTOK + c * 64:g * GTOK + (c + 1) * 64],
                        kT[po:po + 64, h // 2, c * 256:(c + 1) * 256],
                        start=True, stop=True)
                nc.scalar.activation(
                    expb[:, h, :], ps[:], AF.Exp, scale=SCALE,
                    accum_out=sums[:, h:h + 1])
            state[g]["expb"] = expb
            state[g]["sums"] = sums

        def emit_v(g):
            """V projection (token-major)."""
            et_g = state[g]["et"]
            v_g = v_pool.tile([128, 4, HD], BF16, tag="v")
            for m in range(4):
                for n in range(2):
                    ps = ps_proj.tile([128, 512], F32, tag="proj")
                    for k in range(KD):
                        nc.tensor.matmul(
                            ps[:], et_g[:, k, m * 128:(m + 1) * 128],
                            wv_sb[:, k, n * 512:(n + 1) * 512],
                            start=(k == 0), stop=(k == KD - 1))
                    if add_bv:
                        nc.vector.tensor_add(
                            v_g[:, m, n * 512:(n + 1) * 512], ps[:],
                            bv_rep[:, n * 512:(n + 1) * 512])
                    else:
                        nc.scalar.activation(
                            v_g[:, m, n * 512:(n + 1) * 512], ps[:], AF.Copy)
            state[g]["v"] = v_g

        def emit_probnorm(g):
            """Normalize exp by row sums (in place, bf16)."""
            sums = state[g]["sums"]
            expb = state[g]["expb"]
            recip = stats.tile([128, N_HEADS], F32, tag="recip")
            nc.vector.reciprocal(recip[:], sums[:])
            recip_b = stats.tile([128, N_HEADS], BF16, tag="recipb")
            nc.vector.tensor_copy(out=recip_b[:], in_=recip[:])
            nc.vector.tensor_mul(
                expb[:], expb[:],
                recip_b[:].to_broadcast([128, N_HEADS, 256]))

        def emit_tail(g):
            st_g = state.pop(g)
            v_g, expb = st_g["v"], st_g["expb"]

            # probs^T via full-width PE transposes: block (h, f) transposes
            # [128 (2c x i), 128 nj] -> [128 nj, 128 (2c x i)]; 8 blocks
            # share one psum bank (bf16), one copy per bank.
            pT = ptr_pool.tile([128, 32, 128], BF16, tag="pT")
            for blk in range(4):
                ps = ps_tr.tile([128, 8, 128], BF16, tag="tr")
                for s in range(8):
                    idx = blk * 8 + s  # idx = h*2 + f
                    h = idx // 2
                    f = idx % 2
                    nc.tensor.transpose(
                        ps[:, s, :],
                        expb[:, h, f * 128:(f + 1) * 128],
                        ident[:])
                nc.vector.tensor_copy(
                    out=pT[:, blk * 8:(blk + 1) * 8, :], in_=ps[:])

            # attnout^T[dv, i] per head; head pairs share a psum tile.
            aout = ao_pool.tile([128, KD, GTOK], BF16, tag="aout")
            for hp in range(KD):
                ps = ps_av.tile([128, 128], F32, tag="av")
                for c in range(2):
                    for dh in range(2):
                        h = hp * 2 + dh
                        for f in range(2):
                            nc.tensor.matmul(
                                ps[dh * 64:(dh + 1) * 64,
                                   c * 64:(c + 1) * 64],
                                v_g[:, c * 2 + f, h * 64:(h + 1) * 64],
                                pT[:, h * 2 + f, c * 64:(c + 1) * 64],
                                start=(f == 0), stop=(f == 1))
                nc.vector.tensor_copy(out=aout[:, hp, :], in_=ps[:])

            # O projection (token-major) + residual
            for n in range(2):
                ps = ps_proj.tile([128, 512], F32, tag="proj")
                for k in range(KD):
                    nc.tensor.matmul(
                        ps[:], aout[:, k, :],
                        wo_sb[:, k, n * 512:(n + 1) * 512],
                        start=(k == 0), stop=(k == KD - 1))
                outb = out_pool.tile([128, 512], F32, tag="outb")
                nc.vector.tensor_add(
                    outb[:], ps[:], hsb[:, g, n * 512:(n + 1) * 512])
                if add_bo:
                    nc.vector.tensor_add(
                        outb[:], outb[:], bo_rep[:, n * 512:(n + 1) * 512])
                nc.sync.dma_start(
                    out=out_r[:, g, n * 512:(n + 1) * 512], in_=outb[:])

        # group-0 K projection first: it only needs wk + e^T(g0), and fills
        # the PE while h loads and layernorm runs on DVE.
        emit_k(0)

        hsb = p1.tile([128, KD, D_MODEL], F32, tag="hslab")
        for tt in range(KD):
            nc.sync.dma_start(out=hsb[:, tt, :], in_=h_r[:, tt, :])
        wq_sb = consts.tile([128, KD, HD], BF16, tag="wqo")
        nc.sync.dma_start(out=wq_sb[:], in_=wq_r)
        wv_sb = consts.tile([128, KD, HD], BF16, tag="wv")
        nc.sync.dma_start(out=wv_sb[:], in_=wv_r)
        bq_sb = consts.tile([128, KD], F32, tag="bq")
        nc.sync.dma_start(out=bq_sb[:], in_=bq_r)

        # ---- phase 1: layernorm -> hn^T -> Q^T ----
        # Batched stats: one Sqrt activation for all 8 row-tiles keeps the
        # ACT sync-wait count within the walrus per-instruction limit.
        mv = stats.tile([128, KD, 2], F32, tag="mv")
        for tt in range(KD):
            st = stats.tile([128, 2, 6], F32, tag="bnst")
            nc.vector.bn_stats(out=st[:, 0, :], in_=hsb[:, tt, 0:512])
            nc.vector.bn_stats(out=st[:, 1, :], in_=hsb[:, tt, 512:1024])
            nc.vector.bn_aggr(out=mv[:, tt, :], in_=st[:])
        var_eps = stats.tile([128, KD], F32, tag="vareps")
        nc.vector.tensor_scalar_add(
            out=var_eps[:], in0=mv[:, :, 1], scalar1=EPS)
        std = stats.tile([128, KD], F32, tag="std")
        nc.scalar.activation(std[:], var_eps[:], AF.Sqrt)
        rstd = stats.tile([128, KD], F32, tag="rstd")
        nc.vector.reciprocal(rstd[:], std[:])

        hnT = p1.tile([128, KD, TOK], BF16, tag="hnT")
        for tb in range(2):  # 512-token halves
            hn_tiles = []
            for q in range(4):
                tt = tb * 4 + q
                hn_t = hn_pool.tile([128, D_MODEL], BF16, tag="hn")
                nc.vector.tensor_scalar(
                    out=hn_t[:], in0=hsb[:, tt, :],
                    scalar1=mv[:, tt, 0:1], scalar2=rstd[:, tt:tt + 1],
                    op0=ALU.subtract, op1=ALU.mult)
                hn_tiles.append(hn_t)
            for dk in range(KD):
                ps = ps_tr.tile([128, 512], BF16, tag="tr")
                for q in range(4):
                    nc.tensor.transpose(
                        ps[:, q * 128:(q + 1) * 128],
                        hn_tiles[q][:, dk * 128:(dk + 1) * 128],
                        ident[:])
                nc.vector.tensor_copy(
                    out=hnT[:, dk, tb * 512:(tb + 1) * 512], in_=ps[:])

        emit_v(0)

        qT = p1.tile([128, KD, TOK], BF16, tag="qT")
        for m in range(KD):
            for n in range(2):
                ps = ps_proj.tile([128, 512], F32, tag="proj")
                for k in range(KD):
                    nc.tensor.matmul(
                        ps[:], wq_sb[:, k, m * 128:(m + 1) * 128],
                        hnT[:, k, n * 512:(n + 1) * 512],
                        start=(k == 0), stop=(k == KD - 1))
                nc.scalar.activation(
                    qT[:, m, n * 512:(n + 1) * 512], ps[:], AF.Identity,
                    bias=bq_sb[:, m:m + 1])

        # wo reuses wq's slot; DMA waits for the last wq read.
        wo_sb = consts.tile([128, KD, D_MODEL], BF16, tag="wqo")
        nc.sync.dma_start(out=wo_sb[:], in_=wo_r)

        emit_scores(0, with_v=False)
        for g in range(1, NGROUP):
            emit_k(g)
            emit_probnorm(g - 1)
            emit_scores(g, with_v=False)
            emit_v(g)
            emit_tail(g - 1)
        emit_probnorm(NGROUP - 1)
        emit_tail(NGROUP - 1)

    nc.compile()
    return nc


_PROG_CACHE: dict = {}


def _get_program(add_bv: bool, add_bo: bool) -> bass.Bass:
    key = (add_bv, add_bo)
    if key not in _PROG_CACHE:
        _PROG_CACHE[key] = _build(add_bv, add_bo)
    return _PROG_CACHE[key]


def make_in_maps(h, e, Wq, bq, Wk, bk, Wv, bv, Wo, bo, gamma, beta):
    """Host-side sharding/layout prep. Returns (in_maps, add_bv, add_bo)."""
    h = np.asarray(h, dtype=np.float32)
    e = np.asarray(e, dtype=np.float32)
    Wq = np.asarray(Wq, dtype=np.float32)
    bq = np.asarray(bq, dtype=np.float32)
    Wk = np.asarray(Wk, dtype=np.float32)
    bk = np.asarray(bk, dtype=np.float32)
    Wv = np.asarray(Wv, dtype=np.float32)
    bv = np.asarray(bv, dtype=np.float32)
    Wo = np.asarray(Wo, dtype=np.float32)
    bo = np.asarray(bo, dtype=np.float32)
    gamma = np.asarray(gamma, dtype=np.float32)
    beta = np.asarray(beta, dtype=np.float32)

    # fold layernorm affine into the Q projection
    Wq_f = gamma[:, None] * Wq
    bq_f = bq + beta @ Wq

    wq_b = np.ascontiguousarray(Wq_f.astype(NP_BF16))
    wk_b = np.ascontiguousarray(Wk.astype(NP_BF16))
    wv_b = np.ascontiguousarray(Wv.astype(NP_BF16))
    wo_b = np.ascontiguousarray(Wo.astype(NP_BF16))

    add_bv = bool(np.any(bv != 0.0))
    add_bo = bool(np.any(bo != 0.0))

    in_maps = []
    for core in range(N_CORES):
        b = core // 2
        c0 = (core % 2) * CH_PER_CORE
        t0 = c0 * CHUNK_LEN + CHUNK_LEN - 1
        hslab = np.zeros((TOK, D_MODEL), np.float32)
        valid = min(TOK, SEQ - t0)
        hslab[:valid] = h[b, t0:t0 + valid]
        eT = np.ascontiguousarray(
            e[b, c0:c0 + CH_PER_CORE].reshape(NGROUP * KVT, D_MODEL)
            .T.astype(NP_BF16))
        in_maps.append({
            "hslab": hslab,
            "eT": eT,
            "wq": wq_b, "wk": wk_b, "wv": wv_b, "wo": wo_b,
            "bq": bq_f.astype(np.float32), "bk": bk,
            "bv": bv, "bo": bo,
        })
    return in_maps, add_bv, add_bo


def assemble(h, core_outs):
    """Gather per-core [TOK, D_MODEL] slabs into the full output."""
    h = np.asarray(h, dtype=np.float32)
    out = h.copy()
    for core, res in enumerate(core_outs):
        b = core // 2
        c0 = (core % 2) * CH_PER_CORE
        t0 = c0 * CHUNK_LEN + CHUNK_LEN - 1
        valid = min(TOK, SEQ - t0)
        out[b, t0:t0 + valid] = res[:valid]
    return out


def kernel(h, e, Wq, bq, Wk, bk, Wv, bv, Wo, bo, gamma, beta):
    from concourse.bass_utils import run_bass_kernel_spmd

    in_maps, add_bv, add_bo = make_in_maps(
        h, e, Wq, bq, Wk, bk, Wv, bv, Wo, bo, gamma, beta)
    nc = _get_program(add_bv, add_bo)
    res = run_bass_kernel_spmd(nc, in_maps, list(range(N_CORES)))
    core_outs = [r["out"] for r in res.results]
    return assemble(h, core_outs)
